# revision 21
# baseline (speedup 1.0000x reference)
"""Trainium2 Bass kernel for nn_MeshCrossAttention (mesh cross-attention + per-head MLP).

Sharding: data-parallel over batch B=16 -> 2 batches per NeuronCore, 8 cores,
no collectives. All matmul operands are bf16 (PE runs 4x faster than fp32r);
PSUM accumulation stays fp32. Host pre-transposes everything so every matmul
contracts over the partition dim with zero on-device transposes:

  qT  [D, LQ]   = Wq @ query^T        (lhsT = WqT chunk, rhs = queryT chunk)
  kT  [D, LK]   = Wk @ key^T
  v   [LK, D]   = value @ Wv^T        stored head-interleaved [LK, H, HD+1]
                                      with a ones column (denominator row)
  sT  [LK, LQ]  = kT_h^T @ qT_h       (K=64 contraction)
  eT  [LK, LQ]  = exp(sT / 8)         (ScalarE, fused scale, bf16 out)
  cT  [HD+1,LQ] = v_aug^T @ eT        (row HD = softmax denominator)
  r   = 1/denom                        (batched DVE reciprocal, bf16)
  bc  = broadcast r along partitions   (one batched DMA via DRAM per group)
  cn  = cT * bc                        (DVE bf16)
  h1T [HD, LQ]  = gelu(W1T^T @ cn_cat + b1)
  out [LQ, 2HD] = h1T_pair^T @ blockdiag(W2T) + b2

Engine placement: PE matmuls; ScalarE exp+gelu; DVE bias adds, reciprocal,
normalize muls; Pool (gpsimd) PSUM->SBUF context copies + denominator row
extraction. MLP emission for each head-group is deferred into the middle of
the next group's attention so the recip/broadcast latency chain never stalls
the (in-order) PE queue.
"""
import math
import sys

import numpy as np

if "/opt/trn_rl_repo" not in sys.path:
    sys.path.insert(0, "/opt/trn_rl_repo")

import ml_dtypes  # noqa: E402

import concourse.bass as bass  # noqa: E402
import concourse.tile as tile  # noqa: E402
from concourse import bacc, mybir  # noqa: E402
from concourse.bass_utils import run_bass_kernel_spmd  # noqa: E402

F32 = mybir.dt.float32
BF16 = mybir.dt.bfloat16

D, H, HD, J = 1024, 16, 64, 3
B, LQ, LK = 16, 512, 512
P = 128
N_CORES = 8
B_LOC = B // N_CORES  # 2
HG = 4                # heads per normalization group


def _emit(tc, aps):
    nc = tc.nc
    ctx_mgr = []

    def pool(name, bufs, space="SBUF"):
        p = tc.tile_pool(name=name, bufs=bufs, space=space)
        ctx_mgr.append(p)
        return p.__enter__()

    const = pool("const", 1)
    ain = pool("ain", 2)            # [128, 8, 512] bf16 activation blocks
    qt_pool = pool("qt", 4)         # oc-paired [128, 2, 512] bf16
    kt_pool = pool("kt", 12)        # oc-paired [128, 2, 512] bf16
    va_pool = pool("va", 3)         # v_aug [128, 4, H*(HD+1)] bf16
    expp = pool("expp", 4)          # ci-paired exp [128, 2, 512] bf16
    ctxp = pool("ctxp", 24)         # unnormalized ctx [64, 512] bf16
    catp = pool("catp", 3)
    denp = pool("denp", 1)
    bcall = pool("bcall", 1)        # broadcast recip [64, HG, J*LQ] bf16
    h1p = pool("h1p", 2)
    ostg = pool("ostg", 2)          # [128, 4, 128] f32 out staging

    dramp = pool("dramp", 2, "DRAM")      # recip round-trip for DMA broadcast
    ps2 = pool("ps2", 2, "PSUM")          # paired proj/scores [128, 2, 512]
    ps_ctx = pool("ps_ctx", 2, "PSUM")    # [65,512]
    ps_mlp = pool("ps_mlp", 1, "PSUM")    # mlp1 [64,512]
    ps_mlp2 = pool("ps_mlp2", 1, "PSUM")  # mlp2 [128, 4, 128]

    # ---------------- constants (resident weights) ----------------
    def load_w(key):
        t = const.tile([P, 8, D], BF16, tag=key, name=key)
        nc.sync.dma_start(
            out=t[:], in_=aps[key].rearrange("(ic p) d -> p ic d", p=P))
        return t

    wq_sb, wk_sb, wv_sb = load_w("wqt"), load_w("wkt"), load_w("wvt")

    w1t_a = const.tile([P, HD], BF16, tag="w1a", name="w1t_a")
    w1t_b = const.tile([HD, HD], BF16, tag="w1b", name="w1t_b")
    nc.sync.dma_start(out=w1t_a[:], in_=aps["w1t"][0:P, :])
    nc.sync.dma_start(out=w1t_b[:], in_=aps["w1t"][P:J * HD, :])

    w2bd = const.tile([P, P], BF16, tag="w2bd", name="w2bd")
    nc.sync.dma_start(out=w2bd[:], in_=aps["w2bd"][:, :])
    b2bd = const.tile([P, P], F32, tag="b2bd", name="b2bd")
    nc.sync.dma_start(out=b2bd[:], in_=aps["b2bd"][:, :])
    bv_bc = const.tile([P, D], F32, tag="bv_bc", name="bv_bc")
    nc.sync.dma_start(out=bv_bc[:], in_=aps["bv_bc"][:, :])

    bq_sb = const.tile([P, 8], F32, tag="bq", name="bq_sb")
    bk_sb = const.tile([P, 8], F32, tag="bk", name="bk_sb")
    nc.sync.dma_start(out=bq_sb[:], in_=aps["bq"].rearrange("(oc p) -> p oc", p=P))
    nc.sync.dma_start(out=bk_sb[:], in_=aps["bk"].rearrange("(oc p) -> p oc", p=P))
    b1_sb2 = const.tile([P, 1], F32, tag="b1", name="b1_sb2")
    nc.sync.dma_start(out=b1_sb2[0:HD, :], in_=aps["b1"].unsqueeze(1))
    nc.sync.dma_start(out=b1_sb2[HD:P, :], in_=aps["b1"].unsqueeze(1))

    den = denp.tile([97, J * LQ], F32, tag="den", name="den")
    rstage = denp.tile([97, J * LQ], F32, tag="rstage", name="rstage")
    nc.vector.memset(den[:], 1.0)  # unused lanes must stay finite for recip

    def load_acts(ap_slice, name):
        t = ain.tile([P, 8, 512], BF16, tag="ain", name=name)
        nc.sync.dma_start(
            out=t[:], in_=ap_slice.rearrange("(ic p) n -> p ic n", p=P))
        return t

    def proj_pair(w_sb, x_t, bias_sb, op, out_pool, out_tag):
        """Paired output tile [128, 2, 512] for oc chunks (2*op, 2*op+1)."""
        pss = ps2.tile([P, 2, 512], F32, tag="big2", name="pss")
        for sub in range(2):
            oc = 2 * op + sub
            for ic in range(8):
                nc.tensor.matmul(
                    out=pss[:, sub, :], lhsT=w_sb[:, ic, oc * P:(oc + 1) * P],
                    rhs=x_t[:, ic, :], start=(ic == 0), stop=(ic == 7))
        t = out_pool.tile([P, 2, 512], BF16, tag=out_tag, name=out_tag)
        nc.vector.tensor_tensor(
            out=t[:], in0=pss[:],
            in1=bias_sb[:, 2 * op:2 * op + 2].unsqueeze(2).to_broadcast(
                (P, 2, 512)),
            op=mybir.AluOpType.add)
        return t

    def v_unit(x_t, va, nck):
        """Full-D v_aug slice for LK-chunk nck; lhsT shared across D halves
        so consecutive matmuls reuse the stationary tile."""
        pss = ps2.tile([P, 2, 512], F32, tag="big2", name="vss")
        for ic in range(8):
            lhsT = x_t[:, ic, nck * P:(nck + 1) * P]
            nc.tensor.matmul(out=pss[:, 0, :], lhsT=lhsT,
                             rhs=wv_sb[:, ic, 0:512],
                             start=(ic == 0), stop=(ic == 7))
            nc.tensor.matmul(out=pss[:, 1, :], lhsT=lhsT,
                             rhs=wv_sb[:, ic, 512:1024],
                             start=(ic == 0), stop=(ic == 7))
        dst = va[:, nck, :].rearrange("p (h e) -> p h e", e=HD + 1)[:, :, 0:HD]
        nc.vector.tensor_tensor(
            out=dst,
            in0=pss[:].rearrange("p a (h e) -> p (a h) e", e=HD),
            in1=bv_bc[:].rearrange("p (h e) -> p h e", e=HD),
            op=mybir.AluOpType.add)

    pending = []

    def flush_pending(n=1):
        for _ in range(min(n, len(pending))):
            pending.pop(0)()

    def emit_mlp_group(b, heads, ctx_tiles, bc_t):
        """MLP for a whole head group; gelus batched (2 per group, adjacent
        on ScalarE) so the exp<->gelu act-table reload happens once per
        group instead of per head."""
        for pi in range(2):
            ph1 = ps_mlp.tile([P, LQ], F32, tag="mlp1", name="ph1")
            cats = []
            for sub in range(2):
                gi = 2 * pi + sub
                h = heads[gi]
                cat0 = catp.tile([P, LQ], BF16, tag="cat0", name="cat0")
                cat1 = catp.tile([HD, LQ], BF16, tag="cat1", name="cat1")
                for j in range(J):
                    dst = cat0[j * HD:(j + 1) * HD, :] if j < 2 else cat1[:]
                    nc.vector.tensor_mul(
                        dst, ctx_tiles[(j, h)][:],
                        bc_t[:, gi, j * LQ:(j + 1) * LQ])
                cats.append((cat0, cat1))
            for sub in range(2):  # w1t_a stationary reused back-to-back
                nc.tensor.matmul(out=ph1[sub * HD:(sub + 1) * HD, :],
                                 lhsT=w1t_a[:], rhs=cats[sub][0][:],
                                 start=True, stop=False, skip_group_check=True)
            for sub in range(2):
                nc.tensor.matmul(out=ph1[sub * HD:(sub + 1) * HD, :],
                                 lhsT=w1t_b[:], rhs=cats[sub][1][:],
                                 start=False, stop=True, skip_group_check=True)
            h1_pair = h1p.tile([P, LQ], BF16, tag="h1", name="h1_pair")
            nc.scalar.activation(
                out=h1_pair[:], in_=ph1[:],
                func=mybir.ActivationFunctionType.Gelu, bias=b1_sb2[:])
            hp = heads[2 * pi] // 2
            psm = ps_mlp2.tile([P, 4, P], F32, tag="mlp2", name="psm")
            for ncf in range(4):
                nc.tensor.matmul(
                    out=psm[:, ncf, :], lhsT=h1_pair[:, ncf * P:(ncf + 1) * P],
                    rhs=w2bd[:], start=True, stop=True)
            ot = ostg.tile([P, 4, P], F32, tag="ostg", name="ot")
            nc.vector.tensor_tensor(
                out=ot[:], in0=psm[:],
                in1=b2bd.unsqueeze(1).to_broadcast((P, 4, P)),
                op=mybir.AluOpType.add)
            nc.sync.dma_start(
                out=aps["out"][b].rearrange(
                    "(ncf p) d -> p ncf d", p=P)[:, :, hp * P:(hp + 1) * P],
                in_=ot[:])

    for b in range(B_LOC):
        # ================= projections =================
        qx = load_acts(aps["qt_in"][b], "qx")
        qP = [proj_pair(wq_sb, qx, bq_sb, op, qt_pool, "qt") for op in range(4)]

        kP = []
        for j in range(J):
            kx = load_acts(aps["kt_in"][j, b], "kx")
            kP.append([proj_pair(wk_sb, kx, bk_sb, op, kt_pool, "kt")
                       for op in range(4)])

        v_aug = []
        for j in range(J):
            vx = load_acts(aps["vt_in"][j, b], "vx")
            va = va_pool.tile([P, 4, H * (HD + 1)], BF16, tag="va", name="va")
            nc.sync.dma_start(
                out=va[:, :, :].rearrange("p c (h e) -> p c h e", e=HD + 1)[
                    :, :, :, HD],
                in_=aps["ones_cols"][:, :, :])
            for nck in range(4):
                v_unit(vx, va, nck)
            v_aug.append(va)

        # ================= attention + deferred MLP =================
        for hg in range(H // HG):
            heads = list(range(hg * HG, hg * HG + HG))
            ctx_tiles = {}
            for gi, h in enumerate(heads):
                ti, r0 = h // 2, (h % 2) * HD
                kq = qP[ti // 2][:, ti % 2, :]
                for j in range(J):
                    kk = kP[j][ti // 2][:, ti % 2, :]
                    psc = ps_ctx.tile([HD + 1, LQ], F32, tag="ctx", name="psc")
                    for cp in range(2):
                        pss = ps2.tile([P, 2, LQ], F32, tag="big2", name="sss")
                        for sub in range(2):
                            ci = 2 * cp + sub
                            nc.tensor.matmul(
                                out=pss[:, sub, :],
                                lhsT=kk[r0:r0 + HD, ci * P:(ci + 1) * P],
                                rhs=kq[r0:r0 + HD, :],
                                start=True, stop=True)
                        et = expp.tile([P, 2, LQ], BF16, tag="expp", name="et")
                        nc.scalar.activation(
                            out=et[:], in_=pss[:],
                            func=mybir.ActivationFunctionType.Exp,
                            scale=1.0 / math.sqrt(HD))
                        for sub in range(2):
                            ci = 2 * cp + sub
                            va_l = v_aug[j][:, ci, :].rearrange(
                                "p (h e) -> p h e", e=HD + 1)[:, h, :]
                            nc.tensor.matmul(
                                out=psc[:], lhsT=va_l, rhs=et[:, sub, :],
                                start=(ci == 0), stop=(ci == 3))
                    ct = ctxp.tile([HD, LQ], BF16, tag="ctxp", name="ct")
                    nc.vector.tensor_copy(out=ct[:], in_=psc[0:HD, :])
                    nc.vector.tensor_copy(
                        out=den[32 * gi:32 * gi + 1, j * LQ:(j + 1) * LQ],
                        in_=psc[HD:HD + 1, :])
                    ctx_tiles[(j, h)] = ct
                flush_pending(1)  # previous group's MLP, one head per head

            nc.vector.reciprocal_approx_fast(out=rstage[:], in_=den[:])
            rbf = denp.tile([97, J * LQ], BF16, tag="rbf", bufs=1, name="rbf")
            nc.vector.tensor_copy(out=rbf[:], in_=rstage[:])
            dram_r = dramp.tile([HG, J * LQ], BF16, tag="dram_r", name="dram_r")
            for gi in range(HG):
                nc.sync.dma_start(
                    out=dram_r[gi:gi + 1, :],
                    in_=rbf[32 * gi:32 * gi + 1, :])
            bc_t = bcall.tile([HD, HG, J * LQ], BF16, tag="bc", name="bc_t")
            nc.sync.dma_start(
                out=bc_t[:],
                in_=dram_r.unsqueeze(0).to_broadcast((HD, HG, J * LQ)))

            pending.append(
                lambda b_=b, hs_=heads, c_=ctx_tiles, t_=bc_t:
                emit_mlp_group(b_, hs_, c_, t_))

    flush_pending(len(pending))

    for p in reversed(ctx_mgr):
        p.__exit__(None, None, None)


_CACHE = {}


def _build():
    if "nc" in _CACHE:
        return _CACHE["nc"]
    nc = bacc.Bacc("TRN2", target_bir_lowering=False, debug=False)
    shapes = {
        "qt_in": ([B_LOC, D, LQ], BF16),
        "kt_in": ([J, B_LOC, D, LK], BF16),
        "vt_in": ([J, B_LOC, D, LK], BF16),
        "wqt": ([D, D], BF16),
        "wkt": ([D, D], BF16),
        "wvt": ([D, D], BF16),
        "w1t": ([J * HD, HD], BF16),
        "ones_cols": ([P, 4, H], BF16),
        "w2bd": ([P, P], BF16),
        "b2bd": ([P, P], F32),
        "bv_bc": ([P, D], F32),
        "bq": ([D], F32),
        "bk": ([D], F32),
        "b1": ([HD], F32),
    }
    aps = {k: nc.dram_tensor(k, s, dt, kind="ExternalInput").ap()
           for k, (s, dt) in shapes.items()}
    aps["out"] = nc.dram_tensor("out", [B_LOC, LQ, D], F32,
                                kind="ExternalOutput").ap()
    with tile.TileContext(nc) as tc:
        _emit(tc, aps)
    nc.compile()
    _CACHE["nc"] = nc
    return nc


def _prep_in_maps(inputs):
    f32 = np.float32
    bf16 = ml_dtypes.bfloat16
    q = np.ascontiguousarray(np.asarray(inputs["query_states"], f32))
    k = np.ascontiguousarray(np.asarray(inputs["key_states"], f32))
    v = np.ascontiguousarray(np.asarray(inputs["value_states"], f32))
    Wq = np.asarray(inputs["Wq"], f32)
    Wk = np.asarray(inputs["Wk"], f32)
    Wv = np.asarray(inputs["Wv"], f32)
    W1 = np.asarray(inputs["W1"], f32)
    W2 = np.asarray(inputs["W2"], f32)
    bq = np.asarray(inputs["bq"], f32)
    bk = np.asarray(inputs["bk"], f32)
    bv = np.asarray(inputs["bv"], f32)
    b1 = np.asarray(inputs["b1"], f32)
    b2 = np.asarray(inputs["b2"], f32)

    wqt = np.ascontiguousarray(Wq.T).astype(bf16)
    wkt = np.ascontiguousarray(Wk.T).astype(bf16)
    wvt = np.ascontiguousarray(Wv.T).astype(bf16)
    w1t = np.ascontiguousarray(W1.T).astype(bf16)          # [192, 64]
    W2T = W2.T
    w2bd = np.zeros((P, P), f32)
    w2bd[:HD, :HD] = W2T
    w2bd[HD:, HD:] = W2T
    w2bd = w2bd.astype(bf16)
    b2bd = np.tile(np.concatenate([b2, b2]), (P, 1)).astype(f32)
    bv_bc = np.tile(bv, (P, 1)).astype(f32)

    qt_all = np.ascontiguousarray(q.transpose(0, 2, 1)).astype(bf16)
    kt_all = np.ascontiguousarray(k.transpose(0, 1, 3, 2)).astype(bf16)
    vt_all = np.ascontiguousarray(v.transpose(0, 1, 3, 2)).astype(bf16)

    in_maps = []
    for c in range(N_CORES):
        sl = slice(c * B_LOC, (c + 1) * B_LOC)
        in_maps.append({
            "qt_in": np.ascontiguousarray(qt_all[sl]),
            "kt_in": np.ascontiguousarray(kt_all[:, sl]),
            "vt_in": np.ascontiguousarray(vt_all[:, sl]),
            "wqt": wqt, "wkt": wkt, "wvt": wvt,
            "w1t": w1t, "w2bd": w2bd, "b2bd": b2bd, "bv_bc": bv_bc,
            "ones_cols": np.ones((P, 4, H), bf16),
            "bq": bq, "bk": bk, "b1": b1,
        })
    return in_maps


def kernel(**inputs):
    nc = _build()
    in_maps = _prep_in_maps(inputs)
    res = run_bass_kernel_spmd(nc, in_maps, core_ids=list(range(N_CORES)))
    out = np.concatenate([res.results[i]["out"] for i in range(N_CORES)], axis=0)
    return out.astype(np.float32)
